# revision 27
# baseline (speedup 1.0000x reference)
"""Trainium2 Bass kernel for nn_MLPModel_70703751626902 (moe_routing).

Per-robot hypernetwork MLP: each of 1024 samples routes to one of 32
per-robot weight sets (input hypernet 624->256, three 256x256 hidden
layers, output hypernet 256->24).

Strategy (expert-parallel): group samples by robot on the host, shard
robots across the 8 cores (4 robots/core, one per "slot"), so every
core runs dense per-robot matmuls with only its own robots' weights.
Activations stay transposed ([hidden, batch]) the whole way so each
layer's PSUM output feeds the next layer's moving operand directly.

Schedule: all input DMAs ride ONE HWDGE ring (scalar engine) in exact
consumption order -- xm, biases, then one packed [wi|wh|wo] transfer
per slot (slot 0 split so the first matmuls start earlier).  The
tensor engine runs slot-SEQUENTIAL chains (input, 3 hidden layers,
output per slot) so matmul consumption tracks the weight stream and
the tail after the last weight byte is one slot's chain, not a whole
layer sweep.  PSUM->SBUF activations run h0 on vector and h1 on
gpsimd in parallel; per-pair output tiles are stored from the sync
engine (HWDGE) as soon as each pair finishes.

The obs mask and input bias are folded host-side: xm = (obs * mask)
transposed, with seq maskbar rows appended that multiply the bi rows
packed at the bottom of wi.  Hidden-layer biases ride as per-partition
bias operands of the PSUM->SBUF relu ops.

Samples for slot j occupy columns [off_j, off_j + cap_j); robots are
assigned to slots by descending count so padding waste is small.  All
8 cores run an identical program (SPMD).
"""

import numpy as np

F32 = np.float32

# matmul operand dtype: "f32" (exact), "f32r" (tf32-like), "f16"
# (half DMA bytes, full-rate PE, rel err ~3.5e-4), "bf16"
W_DT = "f16"


def _plan(ids, n_robots):
    """Group samples by robot and assign robots to (core, slot)."""
    counts = np.bincount(ids, minlength=n_robots)
    order = np.argsort(-counts, kind="stable")
    n_slots = (n_robots + 7) // 8
    caps = []
    for j in range(n_slots):
        grp = order[8 * j : 8 * j + 8]
        m = int(counts[grp].max()) if len(grp) else 0
        caps.append(max(4, int(np.ceil(max(m, 1) / 4) * 4)))
    offs = np.concatenate([[0], np.cumsum(caps)]).astype(int)
    nb = int(offs[-1])
    assert nb <= 512, f"batch columns per core {nb} exceeds PSUM bank"
    rows = [[None] * n_slots for _ in range(8)]
    robot_at = [[None] * n_slots for _ in range(8)]
    for rank, robot in enumerate(order):
        j, c = rank // 8, rank % 8
        if j >= n_slots:
            break
        rows[c][j] = np.nonzero(ids == robot)[0]
        robot_at[c][j] = int(robot)
    return {
        "caps": tuple(caps),
        "offs": tuple(int(o) for o in offs),
        "nb": nb,
        "rows": rows,
        "robot_at": robot_at,
        "n_slots": n_slots,
    }


def _pack_kp(a, ncols=None):
    """[K, M] -> [128, ceil(K/128)*M]; col kt*M+m holds a[kt*128+p, m]."""
    k, m = a.shape
    nk = (k + 127) // 128
    out = np.zeros((128, nk * m), a.dtype)
    for kt in range(nk):
        ks = min(128, k - kt * 128)
        out[:ks, kt * m : kt * m + m] = a[kt * 128 : kt * 128 + ks, :]
    return out


_PROGRAM_CACHE = {}


def _build_program(caps, kin, seq, hid, kout, w_dt_name):
    import concourse.mybir as mybir
    import concourse.tile as tile
    from concourse import bacc

    f32 = mybir.dt.float32
    wdt = {"f32": f32, "f32r": mybir.dt.float32r, "bf16": mybir.dt.bfloat16,
           "f16": mybir.dt.float16}[w_dt_name]
    n_slots = len(caps)
    offs = np.concatenate([[0], np.cumsum(caps)]).astype(int)
    nb = int(offs[-1])
    # input-layer contraction: obs rows (kin) plus seq maskbar rows that
    # carry the masked input bias (bi rows ride in wi) -- see host prep
    kaug = kin + seq
    nk = (kin + 127) // 128
    assert kaug <= nk * 128, "maskbar fold needs slack in the last chunk"
    klast = kaug - 128 * (nk - 1)
    nh = hid // 128  # hidden column halves
    wiw = nk * hid
    whw = 3 * nh * hid
    wow = nh * kout
    pkw = wiw + whw + wow  # one slot's packed weight columns

    import concourse.bass as bass_mod

    # Skip the framework's init-time all-engine barrier: it only
    # protects the const-AP memsets, which this kernel never reads
    # (every activation bias is a real SBUF column).  All data hazards
    # are still covered by Tile-generated semaphores.
    _orig_barrier = bass_mod.Bass.all_engine_barrier
    bass_mod.Bass.all_engine_barrier = lambda self, *, sem_only=False: None
    try:
        nc = bacc.Bacc("TRN2", target_bir_lowering=False, debug=False, num_devices=8)
    finally:
        bass_mod.Bass.all_engine_barrier = _orig_barrier

    xmw = nk * nb
    bc_d = nc.dram_tensor("bcols", [128, n_slots * 8], f32, kind="ExternalInput")
    # one f16 stream tensor: [xm | wi0 | rest0 | pk1 | ... | wi_last | rest_last]
    pk_d = nc.dram_tensor(
        "pk", [128, xmw + n_slots * pkw], wdt, kind="ExternalInput"
    )
    ot_d = nc.dram_tensor("ot", [kout, nb], wdt, kind="ExternalOutput")

    relu = mybir.ActivationFunctionType.Relu

    with tile.TileContext(nc) as tc:
        with (
            tc.tile_pool(name="sb", bufs=1) as pool,
            tc.tile_pool(name="ps", bufs=6, space="PSUM") as psum,
            tc.tile_pool(name="pso", bufs=2, space="PSUM") as psum_o,
        ):
            # two HWDGE rings: scalar carries the head of the stream in
            # consumption order; sync carries one mid pack in parallel
            # (dual-ring keeps more DMA engines fed) plus the output
            # stores.  Slots 0 and 3 split [wi | rest] so the first
            # matmuls start early and the tail slot's input layer can
            # run while its hidden weights still stream.
            # bc rides the sync ring: it is tiny, so it drains instantly
            # without perturbing the scalar ring's weight stream, and it
            # saves a ~0.6us issue slot at the head of the scalar queue.
            bc_t = pool.tile([128, n_slots * 8], f32, tag="bc")
            nc.sync.dma_start(bc_t[:], bc_d[:, :])

            # head DMA carries xm + slot0's input weights in one transfer
            # (one issue, one completion semaphore for everything the
            # first matmuls need); slot0's hidden weights follow separately
            head = pool.tile([128, xmw + wiw], wdt, tag="head")
            nc.scalar.dma_start(head[:], pk_d[:, 0 : xmw + wiw])
            xm_t = head

            pk_t = [None] * n_slots
            rest0 = pool.tile([128, whw + wow], wdt, tag="rest0")
            nc.scalar.dma_start(
                rest0[:], pk_d[:, xmw + wiw : xmw + pkw]
            )
            pk_t[0] = (head, xmw, rest0, 0)
            for j in range(1, n_slots - 1):
                t = pool.tile([128, pkw], wdt, tag=f"pk{j}")
                nc.scalar.dma_start(
                    t[:], pk_d[:, xmw + j * pkw : xmw + (j + 1) * pkw]
                )
                pk_t[j] = (t, 0, t, wiw)
            if n_slots > 1:
                j = n_slots - 1
                wi_t = pool.tile([128, wiw], wdt, tag=f"wi{j}")
                nc.scalar.dma_start(
                    wi_t[:], pk_d[:, xmw + j * pkw : xmw + j * pkw + wiw]
                )
                r_t = pool.tile([128, whw + wow], wdt, tag=f"rest{j}")
                nc.scalar.dma_start(
                    r_t[:],
                    pk_d[:, xmw + j * pkw + wiw : xmw + (j + 1) * pkw],
                )
                pk_t[j] = (wi_t, 0, r_t, 0)

            def wi_lhsT(j, kt, h, ks):
                t, base, _, _ = pk_t[j]
                o = base + kt * hid + h * 128
                return t[:ks, o : o + 128]

            def wh_lhsT(j, li, pi, h):
                _, _, t, base = pk_t[j]
                o = base + li * nh * hid + pi * hid + h * 128
                return t[:, o : o + 128]

            def wo_lhsT(j, pi):
                _, _, t, base = pk_t[j]
                o = base + whw + pi * kout
                return t[:, o : o + kout]

            zero_bias = bc_t[:, 7:8]  # unused bcols column, always zero
            cmax = max(caps)

            def emit_in(j):
                cap = caps[j]
                o0 = int(offs[j])
                # input layer: accumulate nk chunks into 2 psum halves
                pin = [psum.tile([128, cmax], f32, tag="ps", name=f"i{j}h{h}")
                       for h in range(nh)]
                for kt in range(nk):
                    ks = 128 if kt < nk - 1 else klast
                    rhs = xm_t[:ks, kt * nb + o0 : kt * nb + o0 + cap]
                    for h in range(nh):
                        nc.tensor.matmul(
                            pin[h][:, :cap], wi_lhsT(j, kt, h, ks), rhs,
                            start=(kt == 0), stop=(kt == nk - 1),
                        )
                act = pool.tile([128, nh * cap], wdt, tag=f"a{j}0")
                nc.vector.tensor_scalar(
                    act[:, 0:cap], pin[0][:, :cap], zero_bias, 0.0,
                    mybir.AluOpType.add, mybir.AluOpType.max,
                )
                nc.scalar.activation(
                    act[:, cap : 2 * cap], pin[1][:, :cap], relu, bias=zero_bias,
                )
                return act

            def emit_hidden(j, li, prev):
                cap = caps[j]
                pl = [psum.tile([128, cmax], f32, tag="ps", name=f"l{li}s{j}h{h}")
                      for h in range(nh)]
                for pi in range(nh):
                    rhs = prev[:, pi * cap : (pi + 1) * cap]
                    for h in range(nh):
                        nc.tensor.matmul(
                            pl[h][:, :cap], wh_lhsT(j, li, pi, h), rhs,
                            start=(pi == 0), stop=(pi == nh - 1),
                        )
                nxt = pool.tile([128, nh * cap], wdt, tag=f"a{j}{li + 1}")
                for h in range(nh):
                    bias = bc_t[:, j * 8 + li * 2 + h : j * 8 + li * 2 + h + 1]
                    if h == 0:
                        nc.vector.tensor_scalar(
                            nxt[:, h * cap : (h + 1) * cap], pl[h][:, :cap],
                            bias, 0.0,
                            mybir.AluOpType.add, mybir.AluOpType.max,
                        )
                    else:
                        nc.scalar.activation(
                            nxt[:, h * cap : (h + 1) * cap], pl[h][:, :cap],
                            relu, bias=bias,
                        )
                return nxt

            # output tiles are shared per adjacent-slot pair and stored in
            # one DMA each; stores ride gpsimd's SWDGE queue so sync has
            # no late user work and its exit segment overlaps compute
            ot_tiles = {}
            for p in range(0, n_slots, 2):
                q = min(p + 1, n_slots - 1)
                w = caps[p] + (caps[q] if q != p else 0)
                t = pool.tile([kout, w], wdt, tag=f"ot{p}")
                ot_tiles[p] = (t, 0)
                if q != p:
                    ot_tiles[q] = (t, caps[p])

            def emit_out(j, prev):
                cap = caps[j]
                o0 = int(offs[j])
                po = psum_o.tile([kout, cmax], f32, tag="po", name=f"o{j}")
                for pi in range(nh):
                    nc.tensor.matmul(
                        po[:, :cap], wo_lhsT(j, pi),
                        prev[:, pi * cap : (pi + 1) * cap],
                        start=(pi == 0), stop=(pi == nh - 1),
                    )
                bias = bc_t[:kout, j * 8 + 6 : j * 8 + 7]
                ot_t, oc = ot_tiles[j]
                if j % 2 == 0:
                    nc.vector.tensor_scalar(
                        ot_t[:, oc : oc + cap], po[:, :cap], bias, None,
                        mybir.AluOpType.add,
                    )
                else:
                    nc.scalar.activation(
                        ot_t[:, oc : oc + cap], po[:, :cap],
                        mybir.ActivationFunctionType.Identity, bias=bias,
                    )
                if j % 2 == 1 or j == n_slots - 1:
                    p = j - (j % 2)
                    q = min(p + 1, n_slots - 1)
                    w = caps[p] + (caps[q] if q != p else 0)
                    t, _ = ot_tiles[p]
                    o0p = int(offs[p])
                    nc.gpsimd.dma_start(ot_d[:, o0p : o0p + w], t[:])

            # slots 0..n-3 run as sequential chains (the weight stream is
            # the pacing constraint there anyway); the last two slots'
            # chains are interleaved so the act/semaphore bubbles of one
            # hide behind the other's matmuls -- that pair runs after the
            # stream ends and is the critical tail.
            for j in range(0, n_slots - 2):
                a = emit_in(j)
                for li in range(3):
                    a = emit_hidden(j, li, a)
                emit_out(j, a)
            pair = [j for j in (n_slots - 2, n_slots - 1) if 0 <= j < n_slots]
            pair = sorted(set(pair))
            acts = {}
            for j in pair:
                acts[j] = emit_in(j)
            for li in range(3):
                for j in pair:
                    acts[j] = emit_hidden(j, li, acts[j])
            for j in pair:
                emit_out(j, acts[j])

    nc.compile()
    return nc


def _get_program(caps, kin, seq, hid, kout, w_dt_name):
    key = (caps, kin, seq, hid, kout, w_dt_name)
    if key not in _PROGRAM_CACHE:
        _PROGRAM_CACHE[key] = _build_program(caps, kin, seq, hid, kout, w_dt_name)
    return _PROGRAM_CACHE[key]


def _np_wdt(w_dt_name):
    if w_dt_name == "bf16":
        import ml_dtypes

        return np.dtype(ml_dtypes.bfloat16)
    if w_dt_name == "f16":
        return np.dtype(np.float16)
    return np.dtype(np.float32)


def _prep_core_inputs(plan, c, obs, maskbar, Wi, bi, W1, b1, W2, b2, W3, b3, Wo, bo,
                      w_dt_name):
    seq = maskbar.shape[1]
    kin = obs.shape[1]
    lobs = kin // seq
    hid = Wi.shape[3]
    kout = seq * Wo.shape[3]
    n_slots = plan["n_slots"]
    nb = plan["nb"]
    offs = plan["offs"]
    nk = (kin + 127) // 128
    nh = hid // 128
    wnp = _np_wdt(w_dt_name)
    wiw = nk * hid
    whw = 3 * nh * hid
    wow = nh * kout
    pkw = wiw + whw + wow

    kaug = kin + seq  # obs rows + maskbar rows (carry the input bias)
    xmw = nk * nb
    xm = np.zeros((kaug, nb), F32)
    bc = np.zeros((128, n_slots * 8), F32)
    pk = np.zeros((128, xmw + n_slots * pkw), F32)

    for j in range(n_slots):
        r = plan["robot_at"][c][j]
        if r is None:
            continue
        rows = plan["rows"][c][j]
        n = len(rows)
        o0 = offs[j]
        if n:
            mb = maskbar[rows]
            xm[:kin, o0 : o0 + n] = (obs[rows] * np.repeat(mb, lobs, axis=1)).T
            xm[kin:, o0 : o0 + n] = mb.T
        o2 = xmw + j * pkw
        pk[:, o2 : o2 + wiw] = _pack_kp(
            np.vstack([Wi[r].reshape(kin, hid), bi[r]])
        )
        for li, W in enumerate((W1, W2, W3)):
            pk[:, o2 + wiw + li * nh * hid : o2 + wiw + (li + 1) * nh * hid] = (
                _pack_kp(W[r])
            )
        pk[:, o2 + wiw + whw : o2 + pkw] = _pack_kp(
            Wo[r].transpose(1, 0, 2).reshape(hid, kout)
        )
        for li, bvec in enumerate((b1[r], b2[r], b3[r])):
            for h in range(nh):
                bc[:, j * 8 + li * 2 + h] = bvec[h * 128 : (h + 1) * 128]
        bc[:kout, j * 8 + 6] = bo[r].reshape(-1)

    pk[:, :xmw] = _pack_kp(xm)
    return {
        "bcols": bc,
        "pk": pk.astype(wnp),
    }


def _unshard(plan, results, B, kout):
    out = np.zeros((B, kout), F32)
    offs = plan["offs"]
    for c in range(8):
        ot = results[c]["ot"]
        for j in range(plan["n_slots"]):
            rows = plan["rows"][c][j]
            if rows is None or len(rows) == 0:
                continue
            o0 = offs[j]
            out[rows] = np.asarray(ot[:, o0 : o0 + len(rows)], F32).T
    return out


def kernel(obs, obs_mask, unimal_ids, Wi, bi, W1, b1, W2, b2, W3, b3, Wo, bo,
           _runner=None, _w_dt=None):
    w_dt_name = _w_dt or W_DT
    obs = np.asarray(obs, F32)
    obs_mask = np.asarray(obs_mask)
    ids = np.asarray(unimal_ids).astype(np.int64)
    Wi, bi = np.asarray(Wi, F32), np.asarray(bi, F32)
    W1, b1 = np.asarray(W1, F32), np.asarray(b1, F32)
    W2, b2 = np.asarray(W2, F32), np.asarray(b2, F32)
    W3, b3 = np.asarray(W3, F32), np.asarray(b3, F32)
    Wo, bo = np.asarray(Wo, F32), np.asarray(bo, F32)

    B = obs.shape[0]
    n_robots = Wi.shape[0]
    seq, lobs, hid = Wi.shape[1], Wi.shape[2], Wi.shape[3]
    kin = seq * lobs
    kout = seq * Wo.shape[3]
    maskbar = 1.0 - obs_mask.astype(F32)

    plan = _plan(ids, n_robots)
    nc = _get_program(plan["caps"], kin, seq, hid, kout, w_dt_name)

    in_maps = [
        _prep_core_inputs(plan, c, obs, maskbar, Wi, bi, W1, b1, W2, b2, W3, b3,
                          Wo, bo, w_dt_name)
        for c in range(8)
    ]

    if _runner is None:
        from concourse.bass_utils import run_bass_kernel_spmd

        res = run_bass_kernel_spmd(nc, in_maps, core_ids=list(range(8)))
        results = res.results
    else:
        results = _runner(nc, in_maps)

    return _unshard(plan, results, B, kout)


# revision 28
# speedup vs baseline: 1.0812x; 1.0812x over previous
"""Trainium2 Bass kernel for nn_MLPModel_70703751626902 (moe_routing).

Per-robot hypernetwork MLP: each of 1024 samples routes to one of 32
per-robot weight sets (input hypernet 624->256, three 256x256 hidden
layers, output hypernet 256->24).

Strategy (expert-parallel): group samples by robot on the host, shard
robots across the 8 cores (4 robots/core, one per "slot"), so every
core runs dense per-robot matmuls with only its own robots' weights.
Activations stay transposed ([hidden, batch]) the whole way so each
layer's PSUM output feeds the next layer's moving operand directly.

Schedule: all input DMAs ride ONE HWDGE ring (scalar engine) in exact
consumption order -- xm, biases, then one packed [wi|wh|wo] transfer
per slot (slot 0 split so the first matmuls start earlier).  The
tensor engine runs slot-SEQUENTIAL chains (input, 3 hidden layers,
output per slot) so matmul consumption tracks the weight stream and
the tail after the last weight byte is one slot's chain, not a whole
layer sweep.  PSUM->SBUF activations run h0 on vector and h1 on
gpsimd in parallel; per-pair output tiles are stored from the sync
engine (HWDGE) as soon as each pair finishes.

The obs mask and input bias are folded host-side: xm = (obs * mask)
transposed, with seq maskbar rows appended that multiply the bi rows
packed at the bottom of wi.  Hidden-layer biases ride as per-partition
bias operands of the PSUM->SBUF relu ops.

Samples for slot j occupy columns [off_j, off_j + cap_j); robots are
assigned to slots by descending count so padding waste is small.  All
8 cores run an identical program (SPMD).
"""

import numpy as np

F32 = np.float32

# matmul operand dtype: "f32" (exact), "f32r" (tf32-like), "f16"
# (half DMA bytes, full-rate PE, rel err ~3.5e-4), "bf16"
W_DT = "f16"


def _plan(ids, n_robots):
    """Group samples by robot and assign robots to (core, slot)."""
    counts = np.bincount(ids, minlength=n_robots)
    order = np.argsort(-counts, kind="stable")
    n_slots = (n_robots + 7) // 8
    caps = []
    for j in range(n_slots):
        grp = order[8 * j : 8 * j + 8]
        m = int(counts[grp].max()) if len(grp) else 0
        caps.append(max(4, int(np.ceil(max(m, 1) / 4) * 4)))
    offs = np.concatenate([[0], np.cumsum(caps)]).astype(int)
    nb = int(offs[-1])
    assert nb <= 512, f"batch columns per core {nb} exceeds PSUM bank"
    rows = [[None] * n_slots for _ in range(8)]
    robot_at = [[None] * n_slots for _ in range(8)]
    for rank, robot in enumerate(order):
        j, c = rank // 8, rank % 8
        if j >= n_slots:
            break
        rows[c][j] = np.nonzero(ids == robot)[0]
        robot_at[c][j] = int(robot)
    return {
        "caps": tuple(caps),
        "offs": tuple(int(o) for o in offs),
        "nb": nb,
        "rows": rows,
        "robot_at": robot_at,
        "n_slots": n_slots,
    }


def _pack_kp(a, ncols=None):
    """[K, M] -> [128, ceil(K/128)*M]; col kt*M+m holds a[kt*128+p, m]."""
    k, m = a.shape
    nk = (k + 127) // 128
    out = np.zeros((128, nk * m), a.dtype)
    for kt in range(nk):
        ks = min(128, k - kt * 128)
        out[:ks, kt * m : kt * m + m] = a[kt * 128 : kt * 128 + ks, :]
    return out


_PROGRAM_CACHE = {}


def _build_program(caps, kin, seq, hid, kout, w_dt_name):
    import concourse.mybir as mybir
    import concourse.tile as tile
    from concourse import bacc

    f32 = mybir.dt.float32
    wdt = {"f32": f32, "f32r": mybir.dt.float32r, "bf16": mybir.dt.bfloat16,
           "f16": mybir.dt.float16}[w_dt_name]
    n_slots = len(caps)
    offs = np.concatenate([[0], np.cumsum(caps)]).astype(int)
    nb = int(offs[-1])
    # input-layer contraction: obs rows (kin) plus seq maskbar rows that
    # carry the masked input bias (bi rows ride in wi) -- see host prep
    kaug = kin + seq
    nk = (kin + 127) // 128
    assert kaug <= nk * 128, "maskbar fold needs slack in the last chunk"
    klast = kaug - 128 * (nk - 1)
    nh = hid // 128  # hidden column halves
    wiw = nk * hid
    whw = 3 * nh * hid
    wow = nh * kout
    pkw = wiw + whw + wow  # one slot's packed weight columns

    import concourse.bass as bass_mod

    # Skip the framework's init-time all-engine barrier: it only
    # protects the const-AP memsets, which this kernel never reads
    # (every activation bias is a real SBUF column).  All data hazards
    # are still covered by Tile-generated semaphores.
    _orig_barrier = bass_mod.Bass.all_engine_barrier
    bass_mod.Bass.all_engine_barrier = lambda self, *, sem_only=False: None
    try:
        nc = bacc.Bacc("TRN2", target_bir_lowering=False, debug=False, num_devices=8)
    finally:
        bass_mod.Bass.all_engine_barrier = _orig_barrier

    xmw = nk * nb
    bc_d = nc.dram_tensor("bcols", [128, n_slots * 8], f32, kind="ExternalInput")
    # one f16 stream tensor: [xm | wi0 | rest0 | pk1 | ... | wi_last | rest_last]
    pk_d = nc.dram_tensor(
        "pk", [128, xmw + n_slots * pkw], wdt, kind="ExternalInput"
    )
    ot_d = nc.dram_tensor("ot", [kout, nb], wdt, kind="ExternalOutput")

    relu = mybir.ActivationFunctionType.Relu

    with tile.TileContext(nc) as tc:
        with (
            tc.tile_pool(name="sb", bufs=1) as pool,
            tc.tile_pool(name="ps", bufs=6, space="PSUM") as psum,
            tc.tile_pool(name="pso", bufs=2, space="PSUM") as psum_o,
        ):
            # two HWDGE rings: scalar carries the head of the stream in
            # consumption order; sync carries one mid pack in parallel
            # (dual-ring keeps more DMA engines fed) plus the output
            # stores.  Slots 0 and 3 split [wi | rest] so the first
            # matmuls start early and the tail slot's input layer can
            # run while its hidden weights still stream.
            # bc rides the sync ring: it is tiny, so it drains instantly
            # without perturbing the scalar ring's weight stream, and it
            # saves a ~0.6us issue slot at the head of the scalar queue.
            bc_t = pool.tile([128, n_slots * 8], f32, tag="bc")
            nc.sync.dma_start(bc_t[:], bc_d[:, :])

            # head DMA carries xm + slot0's input weights in one transfer
            # (one issue, one completion semaphore for everything the
            # first matmuls need); slot0's hidden weights follow separately
            head = pool.tile([128, xmw + wiw], wdt, tag="head")
            nc.scalar.dma_start(head[:], pk_d[:, 0 : xmw + wiw])
            xm_t = head

            pk_t = [None] * n_slots
            rest0 = pool.tile([128, whw + wow], wdt, tag="rest0")
            nc.scalar.dma_start(
                rest0[:], pk_d[:, xmw + wiw : xmw + pkw]
            )
            pk_t[0] = (head, xmw, rest0, 0)
            for j in range(1, n_slots - 1):
                t = pool.tile([128, pkw], wdt, tag=f"pk{j}")
                nc.scalar.dma_start(
                    t[:], pk_d[:, xmw + j * pkw : xmw + (j + 1) * pkw]
                )
                pk_t[j] = (t, 0, t, wiw)
            if n_slots > 1:
                j = n_slots - 1
                wi_t = pool.tile([128, wiw], wdt, tag=f"wi{j}")
                nc.scalar.dma_start(
                    wi_t[:], pk_d[:, xmw + j * pkw : xmw + j * pkw + wiw]
                )
                r_t = pool.tile([128, whw + wow], wdt, tag=f"rest{j}")
                nc.scalar.dma_start(
                    r_t[:],
                    pk_d[:, xmw + j * pkw + wiw : xmw + (j + 1) * pkw],
                )
                pk_t[j] = (wi_t, 0, r_t, 0)

            def wi_lhsT(j, kt, h, ks):
                t, base, _, _ = pk_t[j]
                o = base + kt * hid + h * 128
                return t[:ks, o : o + 128]

            def wh_lhsT(j, li, pi, h):
                _, _, t, base = pk_t[j]
                o = base + li * nh * hid + pi * hid + h * 128
                return t[:, o : o + 128]

            def wo_lhsT(j, pi):
                _, _, t, base = pk_t[j]
                o = base + whw + pi * kout
                return t[:, o : o + kout]

            zero_bias = bc_t[:, 7:8]  # unused bcols column, always zero
            cmax = max(caps)

            def emit_in(j):
                cap = caps[j]
                o0 = int(offs[j])
                # input layer: accumulate nk chunks into 2 psum halves
                pin = [psum.tile([128, cmax], f32, tag="ps", name=f"i{j}h{h}")
                       for h in range(nh)]
                for kt in range(nk):
                    ks = 128 if kt < nk - 1 else klast
                    rhs = xm_t[:ks, kt * nb + o0 : kt * nb + o0 + cap]
                    for h in range(nh):
                        nc.tensor.matmul(
                            pin[h][:, :cap], wi_lhsT(j, kt, h, ks), rhs,
                            start=(kt == 0), stop=(kt == nk - 1),
                        )
                act = pool.tile([128, nh * cap], wdt, tag=f"a{j}0")
                nc.vector.tensor_scalar(
                    act[:, 0:cap], pin[0][:, :cap], zero_bias, 0.0,
                    mybir.AluOpType.add, mybir.AluOpType.max,
                )
                nc.scalar.activation(
                    act[:, cap : 2 * cap], pin[1][:, :cap], relu, bias=zero_bias,
                )
                return act

            def emit_hidden(j, li, prev):
                cap = caps[j]
                pl = [psum.tile([128, cmax], f32, tag="ps", name=f"l{li}s{j}h{h}")
                      for h in range(nh)]
                for pi in range(nh):
                    rhs = prev[:, pi * cap : (pi + 1) * cap]
                    for h in range(nh):
                        nc.tensor.matmul(
                            pl[h][:, :cap], wh_lhsT(j, li, pi, h), rhs,
                            start=(pi == 0), stop=(pi == nh - 1),
                        )
                nxt = pool.tile([128, nh * cap], wdt, tag=f"a{j}{li + 1}")
                for h in range(nh):
                    bias = bc_t[:, j * 8 + li * 2 + h : j * 8 + li * 2 + h + 1]
                    if h == 0:
                        nc.vector.tensor_scalar(
                            nxt[:, h * cap : (h + 1) * cap], pl[h][:, :cap],
                            bias, 0.0,
                            mybir.AluOpType.add, mybir.AluOpType.max,
                        )
                    else:
                        nc.scalar.activation(
                            nxt[:, h * cap : (h + 1) * cap], pl[h][:, :cap],
                            relu, bias=bias,
                        )
                return nxt

            def emit_out(j, prev):
                cap = caps[j]
                o0 = int(offs[j])
                po = psum_o.tile([kout, cmax], f32, tag="po", name=f"o{j}")
                for pi in range(nh):
                    nc.tensor.matmul(
                        po[:, :cap], wo_lhsT(j, pi),
                        prev[:, pi * cap : (pi + 1) * cap],
                        start=(pi == 0), stop=(pi == nh - 1),
                    )
                bias = bc_t[:kout, j * 8 + 6 : j * 8 + 7]
                ot_t = pool.tile([kout, cap], wdt, tag=f"ot{j}")
                if j % 2 == 0:
                    nc.vector.tensor_scalar(
                        ot_t[:, :], po[:, :cap], bias, None,
                        mybir.AluOpType.add,
                    )
                else:
                    nc.scalar.activation(
                        ot_t[:, :], po[:, :cap],
                        mybir.ActivationFunctionType.Identity, bias=bias,
                    )
                # stores ride gpsimd's SWDGE queue: sync then has no late
                # user work, so its serial exit-poll segment overlaps
                # compute instead of following the last store
                nc.gpsimd.dma_start(ot_d[:, o0 : o0 + cap], ot_t[:])

            # slots 0..n-3 run as sequential chains (the weight stream is
            # the pacing constraint there anyway); the last two slots'
            # chains are interleaved so the act/semaphore bubbles of one
            # hide behind the other's matmuls -- that pair runs after the
            # stream ends and is the critical tail.
            for j in range(0, n_slots - 2):
                a = emit_in(j)
                for li in range(3):
                    a = emit_hidden(j, li, a)
                emit_out(j, a)
            pair = [j for j in (n_slots - 2, n_slots - 1) if 0 <= j < n_slots]
            pair = sorted(set(pair))
            acts = {}
            for j in pair:
                acts[j] = emit_in(j)
            for li in range(3):
                for j in pair:
                    acts[j] = emit_hidden(j, li, acts[j])
            for j in pair:
                emit_out(j, acts[j])

    nc.compile()
    return nc


def _get_program(caps, kin, seq, hid, kout, w_dt_name):
    key = (caps, kin, seq, hid, kout, w_dt_name)
    if key not in _PROGRAM_CACHE:
        _PROGRAM_CACHE[key] = _build_program(caps, kin, seq, hid, kout, w_dt_name)
    return _PROGRAM_CACHE[key]


def _np_wdt(w_dt_name):
    if w_dt_name == "bf16":
        import ml_dtypes

        return np.dtype(ml_dtypes.bfloat16)
    if w_dt_name == "f16":
        return np.dtype(np.float16)
    return np.dtype(np.float32)


def _prep_core_inputs(plan, c, obs, maskbar, Wi, bi, W1, b1, W2, b2, W3, b3, Wo, bo,
                      w_dt_name):
    seq = maskbar.shape[1]
    kin = obs.shape[1]
    lobs = kin // seq
    hid = Wi.shape[3]
    kout = seq * Wo.shape[3]
    n_slots = plan["n_slots"]
    nb = plan["nb"]
    offs = plan["offs"]
    nk = (kin + 127) // 128
    nh = hid // 128
    wnp = _np_wdt(w_dt_name)
    wiw = nk * hid
    whw = 3 * nh * hid
    wow = nh * kout
    pkw = wiw + whw + wow

    kaug = kin + seq  # obs rows + maskbar rows (carry the input bias)
    xmw = nk * nb
    xm = np.zeros((kaug, nb), F32)
    bc = np.zeros((128, n_slots * 8), F32)
    pk = np.zeros((128, xmw + n_slots * pkw), F32)

    for j in range(n_slots):
        r = plan["robot_at"][c][j]
        if r is None:
            continue
        rows = plan["rows"][c][j]
        n = len(rows)
        o0 = offs[j]
        if n:
            mb = maskbar[rows]
            xm[:kin, o0 : o0 + n] = (obs[rows] * np.repeat(mb, lobs, axis=1)).T
            xm[kin:, o0 : o0 + n] = mb.T
        o2 = xmw + j * pkw
        pk[:, o2 : o2 + wiw] = _pack_kp(
            np.vstack([Wi[r].reshape(kin, hid), bi[r]])
        )
        for li, W in enumerate((W1, W2, W3)):
            pk[:, o2 + wiw + li * nh * hid : o2 + wiw + (li + 1) * nh * hid] = (
                _pack_kp(W[r])
            )
        pk[:, o2 + wiw + whw : o2 + pkw] = _pack_kp(
            Wo[r].transpose(1, 0, 2).reshape(hid, kout)
        )
        for li, bvec in enumerate((b1[r], b2[r], b3[r])):
            for h in range(nh):
                bc[:, j * 8 + li * 2 + h] = bvec[h * 128 : (h + 1) * 128]
        bc[:kout, j * 8 + 6] = bo[r].reshape(-1)

    pk[:, :xmw] = _pack_kp(xm)
    return {
        "bcols": bc,
        "pk": pk.astype(wnp),
    }


def _unshard(plan, results, B, kout):
    out = np.zeros((B, kout), F32)
    offs = plan["offs"]
    for c in range(8):
        ot = results[c]["ot"]
        for j in range(plan["n_slots"]):
            rows = plan["rows"][c][j]
            if rows is None or len(rows) == 0:
                continue
            o0 = offs[j]
            out[rows] = np.asarray(ot[:, o0 : o0 + len(rows)], F32).T
    return out


def kernel(obs, obs_mask, unimal_ids, Wi, bi, W1, b1, W2, b2, W3, b3, Wo, bo,
           _runner=None, _w_dt=None):
    w_dt_name = _w_dt or W_DT
    obs = np.asarray(obs, F32)
    obs_mask = np.asarray(obs_mask)
    ids = np.asarray(unimal_ids).astype(np.int64)
    Wi, bi = np.asarray(Wi, F32), np.asarray(bi, F32)
    W1, b1 = np.asarray(W1, F32), np.asarray(b1, F32)
    W2, b2 = np.asarray(W2, F32), np.asarray(b2, F32)
    W3, b3 = np.asarray(W3, F32), np.asarray(b3, F32)
    Wo, bo = np.asarray(Wo, F32), np.asarray(bo, F32)

    B = obs.shape[0]
    n_robots = Wi.shape[0]
    seq, lobs, hid = Wi.shape[1], Wi.shape[2], Wi.shape[3]
    kin = seq * lobs
    kout = seq * Wo.shape[3]
    maskbar = 1.0 - obs_mask.astype(F32)

    plan = _plan(ids, n_robots)
    nc = _get_program(plan["caps"], kin, seq, hid, kout, w_dt_name)

    in_maps = [
        _prep_core_inputs(plan, c, obs, maskbar, Wi, bi, W1, b1, W2, b2, W3, b3,
                          Wo, bo, w_dt_name)
        for c in range(8)
    ]

    if _runner is None:
        from concourse.bass_utils import run_bass_kernel_spmd

        res = run_bass_kernel_spmd(nc, in_maps, core_ids=list(range(8)))
        results = res.results
    else:
        results = _runner(nc, in_maps)

    return _unshard(plan, results, B, kout)


# revision 29
# speedup vs baseline: 1.1142x; 1.0306x over previous
"""Trainium2 Bass kernel for nn_MLPModel_70703751626902 (moe_routing).

Per-robot hypernetwork MLP: each of 1024 samples routes to one of 32
per-robot weight sets (input hypernet 624->256, three 256x256 hidden
layers, output hypernet 256->24).

Strategy (expert-parallel): group samples by robot on the host, shard
robots across the 8 cores (4 robots/core, one per "slot"), so every
core runs dense per-robot matmuls with only its own robots' weights.
Activations stay transposed ([hidden, batch]) the whole way so each
layer's PSUM output feeds the next layer's moving operand directly.

Schedule: all input DMAs ride ONE HWDGE ring (scalar engine) in exact
consumption order -- xm, biases, then one packed [wi|wh|wo] transfer
per slot (slot 0 split so the first matmuls start earlier).  The
tensor engine runs slot-SEQUENTIAL chains (input, 3 hidden layers,
output per slot) so matmul consumption tracks the weight stream and
the tail after the last weight byte is one slot's chain, not a whole
layer sweep.  PSUM->SBUF activations run h0 on vector and h1 on
gpsimd in parallel; per-pair output tiles are stored from the sync
engine (HWDGE) as soon as each pair finishes.

The obs mask and input bias are folded host-side: xm = (obs * mask)
transposed, with seq maskbar rows appended that multiply the bi rows
packed at the bottom of wi.  Hidden-layer biases ride as per-partition
bias operands of the PSUM->SBUF relu ops.

Samples for slot j occupy columns [off_j, off_j + cap_j); robots are
assigned to slots by descending count so padding waste is small.  All
8 cores run an identical program (SPMD).
"""

import numpy as np

F32 = np.float32

# matmul operand dtype: "f32" (exact), "f32r" (tf32-like), "f16"
# (half DMA bytes, full-rate PE, rel err ~3.5e-4), "bf16"
W_DT = "f16"


def _plan(ids, n_robots):
    """Group samples by robot and assign robots to (core, slot)."""
    counts = np.bincount(ids, minlength=n_robots)
    order = np.argsort(-counts, kind="stable")
    n_slots = (n_robots + 7) // 8
    caps = []
    for j in range(n_slots):
        grp = order[8 * j : 8 * j + 8]
        m = int(counts[grp].max()) if len(grp) else 0
        caps.append(max(4, int(np.ceil(max(m, 1) / 4) * 4)))
    offs = np.concatenate([[0], np.cumsum(caps)]).astype(int)
    nb = int(offs[-1])
    assert nb <= 512, f"batch columns per core {nb} exceeds PSUM bank"
    rows = [[None] * n_slots for _ in range(8)]
    robot_at = [[None] * n_slots for _ in range(8)]
    for rank, robot in enumerate(order):
        j, c = rank // 8, rank % 8
        if j >= n_slots:
            break
        rows[c][j] = np.nonzero(ids == robot)[0]
        robot_at[c][j] = int(robot)
    return {
        "caps": tuple(caps),
        "offs": tuple(int(o) for o in offs),
        "nb": nb,
        "rows": rows,
        "robot_at": robot_at,
        "n_slots": n_slots,
    }


def _pack_kp(a, ncols=None):
    """[K, M] -> [128, ceil(K/128)*M]; col kt*M+m holds a[kt*128+p, m]."""
    k, m = a.shape
    nk = (k + 127) // 128
    out = np.zeros((128, nk * m), a.dtype)
    for kt in range(nk):
        ks = min(128, k - kt * 128)
        out[:ks, kt * m : kt * m + m] = a[kt * 128 : kt * 128 + ks, :]
    return out


_PROGRAM_CACHE = {}


def _build_program(caps, kin, seq, hid, kout, w_dt_name):
    import concourse.mybir as mybir
    import concourse.tile as tile
    from concourse import bacc

    f32 = mybir.dt.float32
    wdt = {"f32": f32, "f32r": mybir.dt.float32r, "bf16": mybir.dt.bfloat16,
           "f16": mybir.dt.float16}[w_dt_name]
    n_slots = len(caps)
    offs = np.concatenate([[0], np.cumsum(caps)]).astype(int)
    nb = int(offs[-1])
    # input-layer contraction: obs rows (kin) plus seq maskbar rows that
    # carry the masked input bias (bi rows ride in wi) -- see host prep
    kaug = kin + seq
    nk = (kin + 127) // 128
    assert kaug <= nk * 128, "maskbar fold needs slack in the last chunk"
    klast = kaug - 128 * (nk - 1)
    nh = hid // 128  # hidden column halves
    wiw = nk * hid
    whw = 3 * nh * hid
    wow = nh * kout
    pkw = wiw + whw + wow  # one slot's packed weight columns

    import concourse.bass as bass_mod

    # Skip the framework's init-time all-engine barrier: it only
    # protects the const-AP memsets, which this kernel never reads
    # (every activation bias is a real SBUF column).  All data hazards
    # are still covered by Tile-generated semaphores.
    _orig_barrier = bass_mod.Bass.all_engine_barrier
    bass_mod.Bass.all_engine_barrier = lambda self, *, sem_only=False: None
    try:
        nc = bacc.Bacc("TRN2", target_bir_lowering=False, debug=False, num_devices=8)
    finally:
        bass_mod.Bass.all_engine_barrier = _orig_barrier

    xmw = nk * nb
    bc_d = nc.dram_tensor("bcols", [128, n_slots * 8], f32, kind="ExternalInput")
    # one f16 stream tensor: [xm | wi0 | rest0 | pk1 | ... | wi_last | rest_last]
    pk_d = nc.dram_tensor(
        "pk", [128, xmw + n_slots * pkw], wdt, kind="ExternalInput"
    )
    ot_d = nc.dram_tensor("ot", [kout, nb], wdt, kind="ExternalOutput")

    relu = mybir.ActivationFunctionType.Relu

    with tile.TileContext(nc) as tc:
        with (
            tc.tile_pool(name="sb", bufs=1) as pool,
            tc.tile_pool(name="ps", bufs=6, space="PSUM") as psum,
            tc.tile_pool(name="pso", bufs=2, space="PSUM") as psum_o,
        ):
            # two HWDGE rings: scalar carries the head of the stream in
            # consumption order; sync carries one mid pack in parallel
            # (dual-ring keeps more DMA engines fed) plus the output
            # stores.  Slots 0 and 3 split [wi | rest] so the first
            # matmuls start early and the tail slot's input layer can
            # run while its hidden weights still stream.
            # bc rides the sync ring: it is tiny, so it drains instantly
            # without perturbing the scalar ring's weight stream, and it
            # saves a ~0.6us issue slot at the head of the scalar queue.
            bc_t = pool.tile([128, n_slots * 8], f32, tag="bc")
            nc.sync.dma_start(bc_t[:], bc_d[:, :])

            # head DMA carries xm + slot0's input weights in one transfer
            # (one issue, one completion semaphore for everything the
            # first matmuls need); slot0's hidden weights follow separately
            head = pool.tile([128, xmw + wiw], wdt, tag="head")
            nc.scalar.dma_start(head[:], pk_d[:, 0 : xmw + wiw])
            xm_t = head

            # pk_t[j]: {"wi": (tile, col), "wh": [(tile, col) per layer],
            #           "wo": (tile, col)}
            pk_t = [None] * n_slots
            lw = nh * hid  # one hidden layer's packed width
            rest0 = pool.tile([128, whw + wow], wdt, tag="rest0")
            nc.scalar.dma_start(
                rest0[:], pk_d[:, xmw + wiw : xmw + pkw]
            )
            pk_t[0] = {
                "wi": (head, xmw),
                "wh": [(rest0, li * lw) for li in range(3)],
                "wo": (rest0, whw),
            }
            for j in range(1, n_slots - 1):
                t = pool.tile([128, pkw], wdt, tag=f"pk{j}")
                nc.scalar.dma_start(
                    t[:], pk_d[:, xmw + j * pkw : xmw + (j + 1) * pkw]
                )
                pk_t[j] = {
                    "wi": (t, 0),
                    "wh": [(t, wiw + li * lw) for li in range(3)],
                    "wo": (t, wiw + whw),
                }
            if n_slots > 1:
                # last slot streams in per-layer pieces so the critical
                # tail chain unblocks as each layer's bytes land
                j = n_slots - 1
                base_d = xmw + j * pkw
                wi_t = pool.tile([128, wiw], wdt, tag=f"wi{j}")
                nc.scalar.dma_start(wi_t[:], pk_d[:, base_d : base_d + wiw])
                whl = []
                for li in range(2):
                    t = pool.tile([128, lw], wdt, tag=f"wh{j}l{li}")
                    nc.scalar.dma_start(
                        t[:],
                        pk_d[:, base_d + wiw + li * lw : base_d + wiw + (li + 1) * lw],
                    )
                    whl.append((t, 0))
                t = pool.tile([128, lw + wow], wdt, tag=f"wh{j}l2")
                nc.scalar.dma_start(
                    t[:], pk_d[:, base_d + wiw + 2 * lw : base_d + pkw]
                )
                whl.append((t, 0))
                pk_t[j] = {"wi": (wi_t, 0), "wh": whl, "wo": (t, lw)}

            def wi_lhsT(j, kt, h, ks):
                t, base = pk_t[j]["wi"]
                o = base + kt * hid + h * 128
                return t[:ks, o : o + 128]

            def wh_lhsT(j, li, pi, h):
                t, base = pk_t[j]["wh"][li]
                o = base + pi * hid + h * 128
                return t[:, o : o + 128]

            def wo_lhsT(j, pi):
                t, base = pk_t[j]["wo"]
                o = base + pi * kout
                return t[:, o : o + kout]

            zero_bias = bc_t[:, 7:8]  # unused bcols column, always zero
            cmax = max(caps)

            def emit_in(j):
                cap = caps[j]
                o0 = int(offs[j])
                # input layer: accumulate nk chunks into 2 psum halves
                pin = [psum.tile([128, cmax], f32, tag="ps", name=f"i{j}h{h}")
                       for h in range(nh)]
                for kt in range(nk):
                    ks = 128 if kt < nk - 1 else klast
                    rhs = xm_t[:ks, kt * nb + o0 : kt * nb + o0 + cap]
                    for h in range(nh):
                        nc.tensor.matmul(
                            pin[h][:, :cap], wi_lhsT(j, kt, h, ks), rhs,
                            start=(kt == 0), stop=(kt == nk - 1),
                        )
                act = pool.tile([128, nh * cap], wdt, tag=f"a{j}0")
                nc.vector.tensor_scalar(
                    act[:, 0:cap], pin[0][:, :cap], zero_bias, 0.0,
                    mybir.AluOpType.add, mybir.AluOpType.max,
                )
                nc.scalar.activation(
                    act[:, cap : 2 * cap], pin[1][:, :cap], relu, bias=zero_bias,
                )
                return act

            def emit_hidden(j, li, prev):
                cap = caps[j]
                pl = [psum.tile([128, cmax], f32, tag="ps", name=f"l{li}s{j}h{h}")
                      for h in range(nh)]
                for pi in range(nh):
                    rhs = prev[:, pi * cap : (pi + 1) * cap]
                    for h in range(nh):
                        nc.tensor.matmul(
                            pl[h][:, :cap], wh_lhsT(j, li, pi, h), rhs,
                            start=(pi == 0), stop=(pi == nh - 1),
                        )
                nxt = pool.tile([128, nh * cap], wdt, tag=f"a{j}{li + 1}")
                for h in range(nh):
                    bias = bc_t[:, j * 8 + li * 2 + h : j * 8 + li * 2 + h + 1]
                    if h == 0:
                        nc.vector.tensor_scalar(
                            nxt[:, h * cap : (h + 1) * cap], pl[h][:, :cap],
                            bias, 0.0,
                            mybir.AluOpType.add, mybir.AluOpType.max,
                        )
                    else:
                        nc.scalar.activation(
                            nxt[:, h * cap : (h + 1) * cap], pl[h][:, :cap],
                            relu, bias=bias,
                        )
                return nxt

            def emit_out(j, prev):
                cap = caps[j]
                o0 = int(offs[j])
                po = psum_o.tile([kout, cmax], f32, tag="po", name=f"o{j}")
                for pi in range(nh):
                    nc.tensor.matmul(
                        po[:, :cap], wo_lhsT(j, pi),
                        prev[:, pi * cap : (pi + 1) * cap],
                        start=(pi == 0), stop=(pi == nh - 1),
                    )
                bias = bc_t[:kout, j * 8 + 6 : j * 8 + 7]
                ot_t = pool.tile([kout, cap], wdt, tag=f"ot{j}")
                if j % 2 == 0:
                    nc.vector.tensor_scalar(
                        ot_t[:, :], po[:, :cap], bias, None,
                        mybir.AluOpType.add,
                    )
                else:
                    nc.scalar.activation(
                        ot_t[:, :], po[:, :cap],
                        mybir.ActivationFunctionType.Identity, bias=bias,
                    )
                # stores ride gpsimd's SWDGE queue: sync then has no late
                # user work, so its serial exit-poll segment overlaps
                # compute instead of following the last store
                nc.gpsimd.dma_start(ot_d[:, o0 : o0 + cap], ot_t[:])

            # slots 0..n-3 run as sequential chains (the weight stream is
            # the pacing constraint there anyway); the last two slots'
            # chains are interleaved so the act/semaphore bubbles of one
            # hide behind the other's matmuls -- that pair runs after the
            # stream ends and is the critical tail.
            for j in range(0, n_slots - 2):
                a = emit_in(j)
                for li in range(3):
                    a = emit_hidden(j, li, a)
                emit_out(j, a)
            pair = [j for j in (n_slots - 2, n_slots - 1) if 0 <= j < n_slots]
            pair = sorted(set(pair))
            acts = {}
            for j in pair:
                acts[j] = emit_in(j)
            for li in range(3):
                for j in pair:
                    acts[j] = emit_hidden(j, li, acts[j])
            for j in pair:
                emit_out(j, acts[j])

    nc.compile()
    return nc


def _get_program(caps, kin, seq, hid, kout, w_dt_name):
    key = (caps, kin, seq, hid, kout, w_dt_name)
    if key not in _PROGRAM_CACHE:
        _PROGRAM_CACHE[key] = _build_program(caps, kin, seq, hid, kout, w_dt_name)
    return _PROGRAM_CACHE[key]


def _np_wdt(w_dt_name):
    if w_dt_name == "bf16":
        import ml_dtypes

        return np.dtype(ml_dtypes.bfloat16)
    if w_dt_name == "f16":
        return np.dtype(np.float16)
    return np.dtype(np.float32)


def _prep_core_inputs(plan, c, obs, maskbar, Wi, bi, W1, b1, W2, b2, W3, b3, Wo, bo,
                      w_dt_name):
    seq = maskbar.shape[1]
    kin = obs.shape[1]
    lobs = kin // seq
    hid = Wi.shape[3]
    kout = seq * Wo.shape[3]
    n_slots = plan["n_slots"]
    nb = plan["nb"]
    offs = plan["offs"]
    nk = (kin + 127) // 128
    nh = hid // 128
    wnp = _np_wdt(w_dt_name)
    wiw = nk * hid
    whw = 3 * nh * hid
    wow = nh * kout
    pkw = wiw + whw + wow

    kaug = kin + seq  # obs rows + maskbar rows (carry the input bias)
    xmw = nk * nb
    xm = np.zeros((kaug, nb), F32)
    bc = np.zeros((128, n_slots * 8), F32)
    pk = np.zeros((128, xmw + n_slots * pkw), F32)

    for j in range(n_slots):
        r = plan["robot_at"][c][j]
        if r is None:
            continue
        rows = plan["rows"][c][j]
        n = len(rows)
        o0 = offs[j]
        if n:
            mb = maskbar[rows]
            xm[:kin, o0 : o0 + n] = (obs[rows] * np.repeat(mb, lobs, axis=1)).T
            xm[kin:, o0 : o0 + n] = mb.T
        o2 = xmw + j * pkw
        pk[:, o2 : o2 + wiw] = _pack_kp(
            np.vstack([Wi[r].reshape(kin, hid), bi[r]])
        )
        for li, W in enumerate((W1, W2, W3)):
            pk[:, o2 + wiw + li * nh * hid : o2 + wiw + (li + 1) * nh * hid] = (
                _pack_kp(W[r])
            )
        pk[:, o2 + wiw + whw : o2 + pkw] = _pack_kp(
            Wo[r].transpose(1, 0, 2).reshape(hid, kout)
        )
        for li, bvec in enumerate((b1[r], b2[r], b3[r])):
            for h in range(nh):
                bc[:, j * 8 + li * 2 + h] = bvec[h * 128 : (h + 1) * 128]
        bc[:kout, j * 8 + 6] = bo[r].reshape(-1)

    pk[:, :xmw] = _pack_kp(xm)
    return {
        "bcols": bc,
        "pk": pk.astype(wnp),
    }


def _unshard(plan, results, B, kout):
    out = np.zeros((B, kout), F32)
    offs = plan["offs"]
    for c in range(8):
        ot = results[c]["ot"]
        for j in range(plan["n_slots"]):
            rows = plan["rows"][c][j]
            if rows is None or len(rows) == 0:
                continue
            o0 = offs[j]
            out[rows] = np.asarray(ot[:, o0 : o0 + len(rows)], F32).T
    return out


def kernel(obs, obs_mask, unimal_ids, Wi, bi, W1, b1, W2, b2, W3, b3, Wo, bo,
           _runner=None, _w_dt=None):
    w_dt_name = _w_dt or W_DT
    obs = np.asarray(obs, F32)
    obs_mask = np.asarray(obs_mask)
    ids = np.asarray(unimal_ids).astype(np.int64)
    Wi, bi = np.asarray(Wi, F32), np.asarray(bi, F32)
    W1, b1 = np.asarray(W1, F32), np.asarray(b1, F32)
    W2, b2 = np.asarray(W2, F32), np.asarray(b2, F32)
    W3, b3 = np.asarray(W3, F32), np.asarray(b3, F32)
    Wo, bo = np.asarray(Wo, F32), np.asarray(bo, F32)

    B = obs.shape[0]
    n_robots = Wi.shape[0]
    seq, lobs, hid = Wi.shape[1], Wi.shape[2], Wi.shape[3]
    kin = seq * lobs
    kout = seq * Wo.shape[3]
    maskbar = 1.0 - obs_mask.astype(F32)

    plan = _plan(ids, n_robots)
    nc = _get_program(plan["caps"], kin, seq, hid, kout, w_dt_name)

    in_maps = [
        _prep_core_inputs(plan, c, obs, maskbar, Wi, bi, W1, b1, W2, b2, W3, b3,
                          Wo, bo, w_dt_name)
        for c in range(8)
    ]

    if _runner is None:
        from concourse.bass_utils import run_bass_kernel_spmd

        res = run_bass_kernel_spmd(nc, in_maps, core_ids=list(range(8)))
        results = res.results
    else:
        results = _runner(nc, in_maps)

    return _unshard(plan, results, B, kout)
